# revision 1
# baseline (speedup 1.0000x reference)
"""GroupedExperts MoE kernel for Trainium2 (8 NeuronCores, expert-parallel).

Reference computation (per expert e):
    h   = x[e] @ W1[e] + b1[e]              # [T, 2D]
    glu = min(h[..., ::2], 7)
    lin = clip(h[..., 1::2], -7, 7)
    s   = glu * sigmoid(1.702 * glu) * (lin + 1)
    out = s @ W2[e] + b2[e]                 # [T, D]

Shapes: E=8, T=2048, D=2048.  One expert per NeuronCore, no cross-core comm.

Device dataflow is fully transposed (features on partitions, tokens on the
free dim) so no on-chip transposes are needed:
    MM1:  hT[f_chunk] = sum_k W1[k, f_chunk].T @ xT[k]     (W1 stationary)
    MM2:  outT[d_chunk] = sum_f W2[f, d_chunk].T @ sT[f]   (W2 stationary)
W1 is de-interleaved on the host into glu/lin halves, so SwiGLU becomes
elementwise between two separate PSUM tiles.  Matmuls run as float32r
(full-rate fp32 mode, 1 col/cycle for moving dim >= 256).
"""

import os
import sys

import numpy as np

for _p in ("/opt/trn_rl_repo", "/root/.axon_site/_ro/trn_rl_repo"):
    if os.path.isdir(_p) and _p not in sys.path:
        sys.path.append(_p)

import concourse.bass as bass  # noqa: E402
import concourse.mybir as mybir  # noqa: E402
import concourse.tile as tile  # noqa: E402
from concourse import bacc  # noqa: E402
from concourse.bass_utils import run_bass_kernel_spmd  # noqa: E402

E = 8
T = 2048
D = 2048
P = 128
KO = D // P      # 16 k-chunks (contraction over D)
FO = D // P      # 16 feature chunks per glu/lin half
DO = D // P      # 16 output-feature chunks
T_TILE = 1024
NT = T // T_TILE  # 2 outer token tiles
NSUB = T_TILE // 512  # 2 psum sub-tiles per token tile

ALPHA = 1.702
LIMIT = 7.0

TRACE = False          # test.py sets True to capture an NTFF profile
LAST_RESULTS = None    # test.py reads exec_time_ns / trace path from here

_CACHE = {}

f32 = mybir.dt.float32
f32r = mybir.dt.float32r


def _emit(tc, xt, w1g, w1l, w2p, b1g, b1l, b2, outT):
    from contextlib import ExitStack

    ctx = ExitStack()
    nc = tc.nc
    Silu = mybir.ActivationFunctionType.Silu
    Ident = mybir.ActivationFunctionType.Identity
    add = mybir.AluOpType.add
    amin = mybir.AluOpType.min
    amax = mybir.AluOpType.max

    const_pool = ctx.enter_context(tc.tile_pool(name="const", bufs=1))
    x_pool = ctx.enter_context(tc.tile_pool(name="xp", bufs=KO))
    s_pool = ctx.enter_context(tc.tile_pool(name="sp", bufs=FO))
    w_pool = ctx.enter_context(tc.tile_pool(name="wp", bufs=5))
    t_pool = ctx.enter_context(tc.tile_pool(name="tp", bufs=2))
    o_pool = ctx.enter_context(tc.tile_pool(name="op", bufs=2))
    ps_pool = ctx.enter_context(tc.tile_pool(name="ps", bufs=8, space="PSUM"))

    b1g_sb = const_pool.tile([P, FO], f32, name="b1g_sb")
    b1l_sb = const_pool.tile([P, FO], f32, name="b1l_sb")
    b2_sb = const_pool.tile([P, DO], f32, name="b2_sb")

    for tt in range(NT):
        t0 = tt * T_TILE
        # f=0 weights are emitted before the xt panel so the first matmul
        # doesn't queue behind the full 8MB panel DMA on the sync queue.
        # For tt=0 the head is fully interleaved (weight k-chunks with panel
        # k-chunks, in consumption order) so the PE starts at ~9us and works
        # while the panel streams in; f=1 weights are queued mid-panel.
        preloaded = {}
        wg0 = w_pool.tile([P, KO, P], f32r, tag="w", name=f"wg_{tt}_0")
        wl0 = w_pool.tile([P, KO, P], f32r, tag="w", name=f"wl_{tt}_0")
        preloaded[0] = (wg0, wl0)
        xts = [
            x_pool.tile([P, T_TILE], f32r, tag="xt", name=f"xt_{tt}_{k}")
            for k in range(KO)
        ]
        if tt == 0:
            for kq in range(0, KO, 4):
                nc.sync.dma_start(
                    wg0[:, kq : kq + 4, :], w1g[0, :, kq : kq + 4, :]
                )
                nc.sync.dma_start(
                    wl0[:, kq : kq + 4, :], w1l[0, :, kq : kq + 4, :]
                )
                for k in range(kq // 2, kq // 2 + 2):
                    nc.sync.dma_start(xts[k][:], xt[k, :, t0 : t0 + T_TILE])
            wg1 = w_pool.tile([P, KO, P], f32r, tag="w", name=f"wg_{tt}_1")
            nc.sync.dma_start(wg1[:], w1g[1])
            wl1 = w_pool.tile([P, KO, P], f32r, tag="w", name=f"wl_{tt}_1")
            nc.sync.dma_start(wl1[:], w1l[1])
            preloaded[1] = (wg1, wl1)
            for k in range(8, KO):
                nc.sync.dma_start(xts[k][:], xt[k, :, t0 : t0 + T_TILE])
            # biases are only needed by the swiglu stage; keep them off the
            # critical head of the DMA queue.
            nc.sync.dma_start(b1g_sb[:], b1g)
            nc.sync.dma_start(b1l_sb[:], b1l)
            nc.sync.dma_start(b2_sb[:], b2)
        else:
            nc.sync.dma_start(wg0[:], w1g[0])
            nc.sync.dma_start(wl0[:], w1l[0])
            for k in range(KO):
                nc.sync.dma_start(xts[k][:], xt[k, :, t0 : t0 + T_TILE])

        s_tiles = [
            s_pool.tile([P, T_TILE], f32r, tag="s", name=f"s_{tt}_{f}")
            for f in range(FO)
        ]

        # ---- MM1 + SwiGLU: sT[f] = swiglu(W1g[:,f].T @ xT, W1l[:,f].T @ xT)
        for f in range(FO):
            if f in preloaded:
                wg, wl = preloaded[f]
            else:
                wg = w_pool.tile([P, KO, P], f32r, tag="w", name=f"wg_{tt}_{f}")
                nc.sync.dma_start(wg[:], w1g[f])
                wl = w_pool.tile([P, KO, P], f32r, tag="w", name=f"wl_{tt}_{f}")
                nc.sync.dma_start(wl[:], w1l[f])

            pg = [
                ps_pool.tile([P, 512], f32, tag="ps", name=f"pg_{tt}_{f}_{ns}")
                for ns in range(NSUB)
            ]
            pl = [
                ps_pool.tile([P, 512], f32, tag="ps", name=f"pl_{tt}_{f}_{ns}")
                for ns in range(NSUB)
            ]
            # pg/pl interleaved per k so each arriving xt chunk unlocks 4
            # matmuls (not 2) — halves PE starvation during the panel fill.
            for k in range(KO):
                for part, lhs in ((pg, wg[:, k, :]), (pl, wl[:, k, :])):
                    for ns in range(NSUB):
                        nc.tensor.matmul(
                            part[ns][:],
                            lhs,
                            xts[k][:, ns * 512 : (ns + 1) * 512],
                            start=(k == 0),
                            stop=(k == KO - 1),
                        )

            for ns in range(NSUB):
                # tg = min(hg + b1g, LIMIT)
                tg = t_pool.tile([P, 512], f32, tag="tg", name=f"tg_{tt}_{f}_{ns}")
                nc.vector.tensor_scalar(
                    out=tg[:],
                    in0=pg[ns][:],
                    scalar1=b1g_sb[:, f : f + 1],
                    scalar2=LIMIT,
                    op0=add,
                    op1=amin,
                )
                # yg = silu(ALPHA*tg) = ALPHA * tg * sigmoid(ALPHA*tg)
                yg = t_pool.tile([P, 512], f32, tag="yg", name=f"yg_{tt}_{f}_{ns}")
                nc.scalar.activation(out=yg[:], in_=tg[:], func=Silu, scale=ALPHA)
                # tl = (clip(hl, -7, 7) + 1)/ALPHA
                #    = clip((hl + b1l + 1)/ALPHA, -6/ALPHA, 8/ALPHA)
                tl = t_pool.tile([P, 512], f32, tag="tl", name=f"tl_{tt}_{f}_{ns}")
                nc.scalar.activation(
                    out=tl[:],
                    in_=pl[ns][:],
                    func=Ident,
                    bias=b1l_sb[:, f : f + 1],
                    scale=1.0 / ALPHA,
                )
                nc.vector.tensor_scalar(
                    out=tl[:],
                    in0=tl[:],
                    scalar1=(LIMIT + 1.0) / ALPHA,
                    scalar2=(-LIMIT + 1.0) / ALPHA,
                    op0=amin,
                    op1=amax,
                )
                # s = yg * tl  (the ALPHA factors cancel)
                nc.vector.tensor_mul(
                    out=s_tiles[f][:, ns * 512 : (ns + 1) * 512],
                    in0=yg[:],
                    in1=tl[:],
                )

        # ---- MM2 + bias: outT[d] = sum_f W2[f, d].T @ sT[f] + b2[d]
        for d in range(DO):
            w2t = w_pool.tile([P, FO, P], f32r, tag="w", name=f"w2_{tt}_{d}")
            nc.sync.dma_start(w2t[:], w2p[d])
            po = [
                ps_pool.tile([P, 512], f32, tag="ps", name=f"po_{tt}_{d}_{ns}")
                for ns in range(NSUB)
            ]
            for ns in range(NSUB):
                for f in range(FO):
                    nc.tensor.matmul(
                        po[ns][:],
                        w2t[:, f, :],
                        s_tiles[f][:, ns * 512 : (ns + 1) * 512],
                        start=(f == 0),
                        stop=(f == FO - 1),
                    )
                ot = o_pool.tile([P, 512], f32, tag="o", name=f"ot_{tt}_{d}_{ns}")
                nc.scalar.activation(
                    out=ot[:], in_=po[ns][:], func=Ident, bias=b2_sb[:, d : d + 1]
                )
                nc.sync.dma_start(
                    outT[d, :, t0 + ns * 512 : t0 + (ns + 1) * 512], ot[:]
                )

    ctx.close()



def _build():
    if "nc" in _CACHE:
        return _CACHE["nc"]
    nc = bacc.Bacc(
        "TRN2",
        target_bir_lowering=False,
        debug=False,
        enable_asserts=False,
        num_devices=E,
    )
    xt = nc.dram_tensor("xt", (KO, P, T), f32r, kind="ExternalInput").ap()
    w1g = nc.dram_tensor("w1g", (FO, P, KO, P), f32r, kind="ExternalInput").ap()
    w1l = nc.dram_tensor("w1l", (FO, P, KO, P), f32r, kind="ExternalInput").ap()
    w2p = nc.dram_tensor("w2p", (DO, P, FO, P), f32r, kind="ExternalInput").ap()
    b1g = nc.dram_tensor("b1g", (P, FO), f32, kind="ExternalInput").ap()
    b1l = nc.dram_tensor("b1l", (P, FO), f32, kind="ExternalInput").ap()
    b2 = nc.dram_tensor("b2", (P, DO), f32, kind="ExternalInput").ap()
    outT = nc.dram_tensor("outT", (DO, P, T), f32, kind="ExternalOutput").ap()
    with tile.TileContext(nc) as tc:
        _emit(tc, xt, w1g, w1l, w2p, b1g, b1l, b2, outT)
    nc.compile()
    _CACHE["nc"] = nc
    return nc


def _pack_w(w):
    # [K, F] -> [fo, p, ko, m] with K = ko*128 + p, F = fo*128 + m
    return np.ascontiguousarray(
        w.reshape(KO, P, FO, P).transpose(2, 1, 0, 3)
    )


def _pack_b(b):
    # [F] -> [p, fo]
    return np.ascontiguousarray(b.reshape(FO, P).T)


def kernel(x, mlp1_weight, mlp1_bias, mlp2_weight, mlp2_bias):
    global LAST_RESULTS
    x = np.asarray(x, np.float32)
    mlp1_weight = np.asarray(mlp1_weight, np.float32)
    mlp1_bias = np.asarray(mlp1_bias, np.float32)
    mlp2_weight = np.asarray(mlp2_weight, np.float32)
    mlp2_bias = np.asarray(mlp2_bias, np.float32)

    nc = _build()
    in_maps = []
    for e in range(E):
        w1 = mlp1_weight[e].reshape(D, 2 * D // 2, 2)  # [K, F, 2] even/odd
        b1 = mlp1_bias[e].reshape(D, 2)
        in_maps.append(
            {
                "xt": np.ascontiguousarray(x[e].T).reshape(KO, P, T),
                "w1g": _pack_w(np.ascontiguousarray(w1[:, :, 0])),
                "w1l": _pack_w(np.ascontiguousarray(w1[:, :, 1])),
                "w2p": _pack_w(mlp2_weight[e]),
                "b1g": _pack_b(np.ascontiguousarray(b1[:, 0])),
                "b1l": _pack_b((np.ascontiguousarray(b1[:, 1]) + 1.0) / ALPHA),
                "b2": _pack_b(mlp2_bias[e]),
            }
        )

    res = run_bass_kernel_spmd(
        nc, in_maps, core_ids=list(range(E)), trace=TRACE
    )
    LAST_RESULTS = res
    out = np.stack(
        [res.results[e]["outT"].reshape(D, T).T for e in range(E)]
    )
    return np.ascontiguousarray(out)



# revision 2
# speedup vs baseline: 1.0153x; 1.0153x over previous
"""GroupedExperts MoE kernel for Trainium2 (8 NeuronCores, expert-parallel).

Reference computation (per expert e):
    h   = x[e] @ W1[e] + b1[e]              # [T, 2D]
    glu = min(h[..., ::2], 7)
    lin = clip(h[..., 1::2], -7, 7)
    s   = glu * sigmoid(1.702 * glu) * (lin + 1)
    out = s @ W2[e] + b2[e]                 # [T, D]

Shapes: E=8, T=2048, D=2048.  One expert per NeuronCore, no cross-core comm.

Device dataflow is fully transposed (features on partitions, tokens on the
free dim) so no on-chip transposes are needed:
    MM1:  hT[f_chunk] = sum_k W1[k, f_chunk].T @ xT[k]     (W1 stationary)
    MM2:  outT[d_chunk] = sum_f W2[f, d_chunk].T @ sT[f]   (W2 stationary)
W1 is de-interleaved on the host into glu/lin halves, so SwiGLU becomes
elementwise between two separate PSUM drains.

All matmul operands are bf16 (rel err ~4e-3, tolerance 2e-2): same PE rate
as float32r (1 col/cycle) but FWL halves LDWEIGHTS, DMA bytes halve, and
the whole T=2048 token range is processed in ONE pass (weights streamed
once).  Per stationary load the PE now runs 4x512-col matmuls.
PSUM: 4 banks per chunk, glu/lin (and consecutive d) chunks double-buffer.
"""

import os
import sys

import numpy as np
import ml_dtypes

for _p in ("/opt/trn_rl_repo", "/root/.axon_site/_ro/trn_rl_repo"):
    if os.path.isdir(_p) and _p not in sys.path:
        sys.path.append(_p)

import concourse.bass as bass  # noqa: E402
import concourse.mybir as mybir  # noqa: E402
import concourse.tile as tile  # noqa: E402
from concourse import bacc  # noqa: E402
from concourse.bass_utils import run_bass_kernel_spmd  # noqa: E402

E = 8
T = 2048
D = 2048
P = 128
KO = D // P      # 16 k-chunks (contraction over D)
FO = D // P      # 16 feature chunks per glu/lin half
DO = D // P      # 16 output-feature chunks
NSUB = T // 512  # 4 psum sub-tiles across the full token range

ALPHA = 1.702
LIMIT = 7.0

TRACE = False          # test.py sets True to capture an NTFF profile
LAST_RESULTS = None    # test.py reads exec_time_ns / trace path from here

_CACHE = {}

f32 = mybir.dt.float32
bf16 = mybir.dt.bfloat16


def _emit(tc, xt, w1g, w1l, w2p, b1g, b1l, b2, outT):
    from contextlib import ExitStack

    ctx = ExitStack()
    nc = tc.nc
    Silu = mybir.ActivationFunctionType.Silu
    Ident = mybir.ActivationFunctionType.Identity
    add = mybir.AluOpType.add
    amin = mybir.AluOpType.min
    amax = mybir.AluOpType.max

    const_pool = ctx.enter_context(tc.tile_pool(name="const", bufs=1))
    x_pool = ctx.enter_context(tc.tile_pool(name="xp", bufs=KO))
    s_pool = ctx.enter_context(tc.tile_pool(name="sp", bufs=FO))
    w_pool = ctx.enter_context(tc.tile_pool(name="wp", bufs=6))
    t_pool = ctx.enter_context(tc.tile_pool(name="tp", bufs=4))
    o_pool = ctx.enter_context(tc.tile_pool(name="op", bufs=8))
    ps_pool = ctx.enter_context(tc.tile_pool(name="ps", bufs=8, space="PSUM"))

    b1g_sb = const_pool.tile([P, FO], f32, name="b1g_sb")
    b1l_sb = const_pool.tile([P, FO], f32, name="b1l_sb")
    b2_sb = const_pool.tile([P, DO], f32, name="b2_sb")

    # ---- DMA head: stream x panel finely, interleaved with f=0 weights in
    # consumption order so the PE starts within ~1us.  Each k iteration of
    # the f=0 double (glu+lin) pass needs wg0[k], wl0[k] (32KB each) and
    # xt[k] (512KB, split in 4 slices): 576KB DMA vs 1.7us of PE work.
    wg0 = w_pool.tile([P, KO, P], bf16, tag="w", name="wg_0")
    wl0 = w_pool.tile([P, KO, P], bf16, tag="w", name="wl_0")
    xts = [x_pool.tile([P, T], bf16, tag="xt", name=f"xt_{k}") for k in range(KO)]
    for k in range(KO):
        nc.sync.dma_start(wg0[:, k, :], w1g[0, :, k, :])
        nc.sync.dma_start(wl0[:, k, :], w1l[0, :, k, :])
        for c in range(0, T, 512):
            nc.sync.dma_start(xts[k][:, c : c + 512], xt[k, :, c : c + 512])
        if k == 0:
            # biases ride early but off the absolute head
            nc.sync.dma_start(b1g_sb[:], b1g)
            nc.sync.dma_start(b1l_sb[:], b1l)
            nc.sync.dma_start(b2_sb[:], b2)

    s_tiles = [
        s_pool.tile([P, T], bf16, tag="s", name=f"s_{f}") for f in range(FO)
    ]

    def swiglu_glu(pg, f):
        # s[f] = Silu(ALPHA * min(pg + b1g, LIMIT))  (per 512-col subtile)
        for ns in range(NSUB):
            tg = t_pool.tile([P, 512], f32, tag="t", name=f"tg_{f}_{ns}")
            nc.vector.tensor_scalar(
                out=tg[:],
                in0=pg[ns][:],
                scalar1=b1g_sb[:, f : f + 1],
                scalar2=LIMIT,
                op0=add,
                op1=amin,
            )
            nc.scalar.activation(
                out=s_tiles[f][:, ns * 512 : (ns + 1) * 512],
                in_=tg[:],
                func=Silu,
                scale=ALPHA,
            )

    def swiglu_lin(pl, f):
        # s[f] *= clip((pl + b1l + 1)/ALPHA, (1-LIMIT)/ALPHA, (1+LIMIT)/ALPHA)
        for ns in range(NSUB):
            tl = t_pool.tile([P, 512], f32, tag="t", name=f"tl_{f}_{ns}")
            nc.scalar.activation(
                out=tl[:],
                in_=pl[ns][:],
                func=Ident,
                bias=b1l_sb[:, f : f + 1],
                scale=1.0 / ALPHA,
            )
            nc.vector.tensor_scalar(
                out=tl[:],
                in0=tl[:],
                scalar1=(LIMIT + 1.0) / ALPHA,
                scalar2=(-LIMIT + 1.0) / ALPHA,
                op0=amin,
                op1=amax,
            )
            sl = s_tiles[f][:, ns * 512 : (ns + 1) * 512]
            nc.vector.tensor_mul(out=sl, in0=sl, in1=tl[:])

    # ---- f = 0: glu+lin interleaved per k so each arriving x chunk feeds
    # 8 matmuls (1.7us PE vs 1.6us DMA) -- PE chases the panel fill.
    pg = [ps_pool.tile([P, 512], f32, tag="ps", name=f"pg_0_{ns}") for ns in range(NSUB)]
    pl = [ps_pool.tile([P, 512], f32, tag="ps", name=f"pl_0_{ns}") for ns in range(NSUB)]
    for k in range(KO):
        for part, w in ((pg, wg0), (pl, wl0)):
            for ns in range(NSUB):
                nc.tensor.matmul(
                    part[ns][:],
                    w[:, k, :],
                    xts[k][:, ns * 512 : (ns + 1) * 512],
                    start=(k == 0),
                    stop=(k == KO - 1),
                )
    swiglu_glu(pg, 0)
    swiglu_lin(pl, 0)

    # ---- f >= 1: separate glu / lin chunk passes, 4 PSUM banks each,
    # so one chunk drains while the next computes.
    for f in range(1, FO):
        for half, src in (("g", w1g), ("l", w1l)):
            wt = w_pool.tile([P, KO, P], bf16, tag="w", name=f"w{half}_{f}")
            nc.sync.dma_start(wt[:], src[f])
            ps = [
                ps_pool.tile([P, 512], f32, tag="ps", name=f"p{half}_{f}_{ns}")
                for ns in range(NSUB)
            ]
            for k in range(KO):
                for ns in range(NSUB):
                    nc.tensor.matmul(
                        ps[ns][:],
                        wt[:, k, :],
                        xts[k][:, ns * 512 : (ns + 1) * 512],
                        start=(k == 0),
                        stop=(k == KO - 1),
                    )
            if half == "g":
                swiglu_glu(ps, f)
            else:
                swiglu_lin(ps, f)

    # ---- MM2 + bias: outT[d] = sum_f W2[f, d].T @ sT[f] + b2[d]
    for d in range(DO):
        w2t = w_pool.tile([P, FO, P], bf16, tag="w", name=f"w2_{d}")
        nc.sync.dma_start(w2t[:], w2p[d])
        po = [
            ps_pool.tile([P, 512], f32, tag="ps", name=f"po_{d}_{ns}")
            for ns in range(NSUB)
        ]
        for f in range(FO):
            for ns in range(NSUB):
                nc.tensor.matmul(
                    po[ns][:],
                    w2t[:, f, :],
                    s_tiles[f][:, ns * 512 : (ns + 1) * 512],
                    start=(f == 0),
                    stop=(f == FO - 1),
                )
        for ns in range(NSUB):
            ot = o_pool.tile([P, 512], f32, tag="o", name=f"ot_{d}_{ns}")
            nc.scalar.activation(
                out=ot[:], in_=po[ns][:], func=Ident, bias=b2_sb[:, d : d + 1]
            )
            nc.sync.dma_start(
                outT[d, :, ns * 512 : (ns + 1) * 512], ot[:]
            )

    ctx.close()


def _build():
    if "nc" in _CACHE:
        return _CACHE["nc"]
    nc = bacc.Bacc(
        "TRN2",
        target_bir_lowering=False,
        debug=False,
        enable_asserts=False,
        num_devices=E,
    )
    xt = nc.dram_tensor("xt", (KO, P, T), bf16, kind="ExternalInput").ap()
    w1g = nc.dram_tensor("w1g", (FO, P, KO, P), bf16, kind="ExternalInput").ap()
    w1l = nc.dram_tensor("w1l", (FO, P, KO, P), bf16, kind="ExternalInput").ap()
    w2p = nc.dram_tensor("w2p", (DO, P, FO, P), bf16, kind="ExternalInput").ap()
    b1g = nc.dram_tensor("b1g", (P, FO), f32, kind="ExternalInput").ap()
    b1l = nc.dram_tensor("b1l", (P, FO), f32, kind="ExternalInput").ap()
    b2 = nc.dram_tensor("b2", (P, DO), f32, kind="ExternalInput").ap()
    outT = nc.dram_tensor("outT", (DO, P, T), f32, kind="ExternalOutput").ap()
    with tile.TileContext(nc) as tc:
        _emit(tc, xt, w1g, w1l, w2p, b1g, b1l, b2, outT)
    nc.compile()
    _CACHE["nc"] = nc
    return nc


def _pack_w(w):
    # [K, F] -> [fo, p, ko, m] with K = ko*128 + p, F = fo*128 + m
    return np.ascontiguousarray(
        w.reshape(KO, P, FO, P).transpose(2, 1, 0, 3).astype(ml_dtypes.bfloat16)
    )


def _pack_b(b):
    # [F] -> [p, fo]
    return np.ascontiguousarray(b.reshape(FO, P).T)


def kernel(x, mlp1_weight, mlp1_bias, mlp2_weight, mlp2_bias):
    global LAST_RESULTS
    x = np.asarray(x, np.float32)
    mlp1_weight = np.asarray(mlp1_weight, np.float32)
    mlp1_bias = np.asarray(mlp1_bias, np.float32)
    mlp2_weight = np.asarray(mlp2_weight, np.float32)
    mlp2_bias = np.asarray(mlp2_bias, np.float32)

    nc = _build()
    in_maps = []
    for e in range(E):
        w1 = mlp1_weight[e].reshape(D, 2 * D // 2, 2)  # [K, F, 2] even/odd
        b1 = mlp1_bias[e].reshape(D, 2)
        in_maps.append(
            {
                "xt": np.ascontiguousarray(
                    x[e].T.astype(ml_dtypes.bfloat16)
                ).reshape(KO, P, T),
                "w1g": _pack_w(np.ascontiguousarray(w1[:, :, 0])),
                "w1l": _pack_w(np.ascontiguousarray(w1[:, :, 1])),
                "w2p": _pack_w(mlp2_weight[e]),
                "b1g": _pack_b(np.ascontiguousarray(b1[:, 0])),
                "b1l": _pack_b((np.ascontiguousarray(b1[:, 1]) + 1.0) / ALPHA),
                "b2": _pack_b(mlp2_bias[e]),
            }
        )

    res = run_bass_kernel_spmd(
        nc, in_maps, core_ids=list(range(E)), trace=TRACE
    )
    LAST_RESULTS = res
    out = np.stack(
        [res.results[e]["outT"].reshape(D, T).T for e in range(E)]
    )
    return np.ascontiguousarray(out)


# revision 6
# speedup vs baseline: 1.0666x; 1.0506x over previous
"""GroupedExperts MoE kernel for Trainium2 (8 NeuronCores, expert-parallel).

Reference computation (per expert e):
    h   = x[e] @ W1[e] + b1[e]              # [T, 2D]
    glu = min(h[..., ::2], 7)
    lin = clip(h[..., 1::2], -7, 7)
    s   = glu * sigmoid(1.702 * glu) * (lin + 1)
    out = s @ W2[e] + b2[e]                 # [T, D]

Shapes: E=8, T=2048, D=2048.  One expert per NeuronCore, no cross-core comm.

Device dataflow is fully transposed (features on partitions, tokens on the
free dim) so no on-chip transposes are needed:
    MM1:  hT[f_chunk] = sum_k W1[k, f_chunk].T @ xT[k]     (W1 stationary)
    MM2:  outT[d_chunk] = sum_f W2[f, d_chunk].T @ sT[f]   (W2 stationary)
W1 is de-interleaved on the host into glu/lin halves, so SwiGLU becomes
elementwise between two separate PSUM drains.

All matmul operands are bf16 (rel err ~4e-3, tolerance 2e-2): same PE rate
as float32r (1 col/cycle) but FWL halves LDWEIGHTS, DMA bytes halve, and
the whole T=2048 token range is processed in ONE pass (weights streamed
once).  Per stationary load the PE now runs 4x512-col matmuls.
PSUM: 4 banks per chunk, glu/lin (and consecutive d) chunks double-buffer.
"""

import os
import sys

import numpy as np
import ml_dtypes

for _p in ("/opt/trn_rl_repo", "/root/.axon_site/_ro/trn_rl_repo"):
    if os.path.isdir(_p) and _p not in sys.path:
        sys.path.append(_p)

import concourse.bass as bass  # noqa: E402
import concourse.mybir as mybir  # noqa: E402
import concourse.tile as tile  # noqa: E402
from concourse import bacc  # noqa: E402
from concourse.bass_utils import run_bass_kernel_spmd  # noqa: E402

E = 8
T = 2048
D = 2048
P = 128
KO = D // P      # 16 k-chunks (contraction over D)
FO = D // P      # 16 feature chunks per glu/lin half
DO = D // P      # 16 output-feature chunks
NSUB = T // 512  # 4 psum sub-tiles across the full token range

ALPHA = 1.702
LIMIT = 7.0

TRACE = False          # test.py sets True to capture an NTFF profile
LAST_RESULTS = None    # test.py reads exec_time_ns / trace path from here

_CACHE = {}

f32 = mybir.dt.float32
bf16 = mybir.dt.bfloat16


def _emit(tc, xt, w1g, w1l, w2p, b1g, b1l, b2, outT):
    from contextlib import ExitStack

    ctx = ExitStack()
    nc = tc.nc
    Silu = mybir.ActivationFunctionType.Silu
    Ident = mybir.ActivationFunctionType.Identity
    add = mybir.AluOpType.add
    amin = mybir.AluOpType.min
    amax = mybir.AluOpType.max

    const_pool = ctx.enter_context(tc.tile_pool(name="const", bufs=1))
    x_pool = ctx.enter_context(tc.tile_pool(name="xp", bufs=KO))
    s_pool = ctx.enter_context(tc.tile_pool(name="sp", bufs=FO))
    w_pool = ctx.enter_context(tc.tile_pool(name="wp", bufs=6))
    t_pool = ctx.enter_context(tc.tile_pool(name="tp", bufs=4))
    o_pool = ctx.enter_context(tc.tile_pool(name="op", bufs=2))
    ps_pool = ctx.enter_context(tc.tile_pool(name="ps", bufs=8, space="PSUM"))

    b1g_sb = const_pool.tile([P, FO], f32, name="b1g_sb")
    b1l_sb = const_pool.tile([P, FO], f32, name="b1l_sb")
    b2_sb = const_pool.tile([P, DO], f32, name="b2_sb")

    # ---- DMA head.  Issuing a [128, N] DMA costs ~0.6us of queue occupancy
    # (128 descriptors) regardless of N, so use FEW, WHOLE-TILE transfers:
    # x panel = 16 DMAs on the sync queue; all weights ride the scalar
    # (Activation) HWDGE queue so they never queue behind the panel.
    wg0 = w_pool.tile([P, KO, P], bf16, tag="w", name="wg_0")
    wl0 = w_pool.tile([P, KO, P], bf16, tag="w", name="wl_0")
    xts = [x_pool.tile([P, T], bf16, tag="xt", name=f"xt_{k}") for k in range(KO)]
    nc.scalar.dma_start(wg0[:], w1g[0])
    nc.scalar.dma_start(wl0[:], w1l[0])
    nc.scalar.dma_start(b1g_sb[:], b1g)
    nc.scalar.dma_start(b1l_sb[:], b1l)
    nc.scalar.dma_start(b2_sb[:], b2)
    for k in range(KO):
        nc.sync.dma_start(xts[k][:], xt[k])

    s_tiles = [
        s_pool.tile([P, T], bf16, tag="s", name=f"s_{f}") for f in range(FO)
    ]

    def swiglu_glu(pg, f):
        # s[f] = Silu(ALPHA * min(pg + b1g, LIMIT))  (per 512-col subtile)
        for ns in range(NSUB):
            tg = t_pool.tile([P, 512], f32, tag="t", name=f"tg_{f}_{ns}")
            nc.vector.tensor_scalar(
                out=tg[:],
                in0=pg[ns][:],
                scalar1=b1g_sb[:, f : f + 1],
                scalar2=LIMIT,
                op0=add,
                op1=amin,
            )
            nc.scalar.activation(
                out=s_tiles[f][:, ns * 512 : (ns + 1) * 512],
                in_=tg[:],
                func=Silu,
                scale=ALPHA,
            )

    def swiglu_lin(pl, f):
        # s[f] *= clip((pl + b1l + 1)/ALPHA, (1-LIMIT)/ALPHA, (1+LIMIT)/ALPHA)
        for ns in range(NSUB):
            tl = t_pool.tile([P, 512], f32, tag="t", name=f"tl_{f}_{ns}")
            nc.scalar.activation(
                out=tl[:],
                in_=pl[ns][:],
                func=Ident,
                bias=b1l_sb[:, f : f + 1],
                scale=1.0 / ALPHA,
            )
            nc.vector.tensor_scalar(
                out=tl[:],
                in0=tl[:],
                scalar1=(LIMIT + 1.0) / ALPHA,
                scalar2=(-LIMIT + 1.0) / ALPHA,
                op0=amin,
                op1=amax,
            )
            sl = s_tiles[f][:, ns * 512 : (ns + 1) * 512]
            nc.vector.tensor_mul(out=sl, in0=sl, in1=tl[:])

    # ---- f = 0: glu+lin interleaved per k so each arriving x chunk feeds
    # 8 matmuls (1.7us PE vs 1.6us DMA) -- PE chases the panel fill.
    pg = [ps_pool.tile([P, 512], f32, tag="ps", name=f"pg_0_{ns}") for ns in range(NSUB)]
    pl = [ps_pool.tile([P, 512], f32, tag="ps", name=f"pl_0_{ns}") for ns in range(NSUB)]
    for k in range(KO):
        for part, w in ((pg, wg0), (pl, wl0)):
            for ns in range(NSUB):
                nc.tensor.matmul(
                    part[ns][:],
                    w[:, k, :],
                    xts[k][:, ns * 512 : (ns + 1) * 512],
                    start=(k == 0),
                    stop=(k == KO - 1),
                )
    swiglu_glu(pg, 0)
    swiglu_lin(pl, 0)

    # ---- f >= 1: separate glu / lin chunk passes, 4 PSUM banks each,
    # so one chunk drains while the next computes.
    for f in range(1, FO):
        for half, src in (("g", w1g), ("l", w1l)):
            wt = w_pool.tile([P, KO, P], bf16, tag="w", name=f"w{half}_{f}")
            nc.scalar.dma_start(wt[:], src[f])
            ps = [
                ps_pool.tile([P, 512], f32, tag="ps", name=f"p{half}_{f}_{ns}")
                for ns in range(NSUB)
            ]
            for k in range(KO):
                for ns in range(NSUB):
                    nc.tensor.matmul(
                        ps[ns][:],
                        wt[:, k, :],
                        xts[k][:, ns * 512 : (ns + 1) * 512],
                        start=(k == 0),
                        stop=(k == KO - 1),
                    )
            if half == "g":
                swiglu_glu(ps, f)
            else:
                swiglu_lin(ps, f)

    # ---- MM2 + bias: outT[d] = sum_f W2[f, d].T @ sT[f] + b2[d]
    for d in range(DO):
        w2t = w_pool.tile([P, FO, P], bf16, tag="w", name=f"w2_{d}")
        nc.scalar.dma_start(w2t[:], w2p[d])
        po = [
            ps_pool.tile([P, 512], f32, tag="ps", name=f"po_{d}_{ns}")
            for ns in range(NSUB)
        ]
        for f in range(FO):
            for ns in range(NSUB):
                nc.tensor.matmul(
                    po[ns][:],
                    w2t[:, f, :],
                    s_tiles[f][:, ns * 512 : (ns + 1) * 512],
                    start=(f == 0),
                    stop=(f == FO - 1),
                )
        ot = o_pool.tile([P, T], f32, tag="o", name=f"ot_{d}")
        for ns in range(NSUB):
            nc.scalar.activation(
                out=ot[:, ns * 512 : (ns + 1) * 512],
                in_=po[ns][:],
                func=Ident,
                bias=b2_sb[:, d : d + 1],
            )
        nc.sync.dma_start(outT[d], ot[:])

    ctx.close()


def _build():
    if "nc" in _CACHE:
        return _CACHE["nc"]
    nc = bacc.Bacc(
        "TRN2",
        target_bir_lowering=False,
        debug=False,
        enable_asserts=False,
        num_devices=E,
    )
    xt = nc.dram_tensor("xt", (KO, P, T), bf16, kind="ExternalInput").ap()
    w1g = nc.dram_tensor("w1g", (FO, P, KO, P), bf16, kind="ExternalInput").ap()
    w1l = nc.dram_tensor("w1l", (FO, P, KO, P), bf16, kind="ExternalInput").ap()
    w2p = nc.dram_tensor("w2p", (DO, P, FO, P), bf16, kind="ExternalInput").ap()
    b1g = nc.dram_tensor("b1g", (P, FO), f32, kind="ExternalInput").ap()
    b1l = nc.dram_tensor("b1l", (P, FO), f32, kind="ExternalInput").ap()
    b2 = nc.dram_tensor("b2", (P, DO), f32, kind="ExternalInput").ap()
    outT = nc.dram_tensor("outT", (DO, P, T), f32, kind="ExternalOutput").ap()
    with tile.TileContext(nc) as tc:
        _emit(tc, xt, w1g, w1l, w2p, b1g, b1l, b2, outT)
    nc.compile()
    _CACHE["nc"] = nc
    return nc


def _pack_w(w):
    # [K, F] -> [fo, p, ko, m] with K = ko*128 + p, F = fo*128 + m
    return np.ascontiguousarray(
        w.reshape(KO, P, FO, P).transpose(2, 1, 0, 3).astype(ml_dtypes.bfloat16)
    )


def _pack_b(b):
    # [F] -> [p, fo]
    return np.ascontiguousarray(b.reshape(FO, P).T)


def kernel(x, mlp1_weight, mlp1_bias, mlp2_weight, mlp2_bias):
    global LAST_RESULTS
    x = np.asarray(x, np.float32)
    mlp1_weight = np.asarray(mlp1_weight, np.float32)
    mlp1_bias = np.asarray(mlp1_bias, np.float32)
    mlp2_weight = np.asarray(mlp2_weight, np.float32)
    mlp2_bias = np.asarray(mlp2_bias, np.float32)

    nc = _build()
    in_maps = []
    for e in range(E):
        w1 = mlp1_weight[e].reshape(D, 2 * D // 2, 2)  # [K, F, 2] even/odd
        b1 = mlp1_bias[e].reshape(D, 2)
        in_maps.append(
            {
                "xt": np.ascontiguousarray(
                    x[e].T.astype(ml_dtypes.bfloat16)
                ).reshape(KO, P, T),
                "w1g": _pack_w(np.ascontiguousarray(w1[:, :, 0])),
                "w1l": _pack_w(np.ascontiguousarray(w1[:, :, 1])),
                "w2p": _pack_w(mlp2_weight[e]),
                "b1g": _pack_b(np.ascontiguousarray(b1[:, 0])),
                "b1l": _pack_b((np.ascontiguousarray(b1[:, 1]) + 1.0) / ALPHA),
                "b2": _pack_b(mlp2_bias[e]),
            }
        )

    res = run_bass_kernel_spmd(
        nc, in_maps, core_ids=list(range(E)), trace=TRACE
    )
    LAST_RESULTS = res
    out = np.stack(
        [res.results[e]["outT"].reshape(D, T).T for e in range(E)]
    )
    return np.ascontiguousarray(out)
